# revision 79
# baseline (speedup 1.0000x reference)
"""Causal multi-head attention on 8 Trainium2 NeuronCores.

Sharding: core c -> (batch g = c // 4, head-group p = c % 4, heads 4p..4p+3).
Each core projects Q/K/V for its batch with its 256 feature columns
(column-sharded w_q/w_k/w_v), runs causal attention for its 4 heads, computes
the partial output projection with its 256 rows of w_o, and a ReduceScatter
over each batch group sums the partials.

All matmul operands are bf16 (fp32 PSUM accumulation; the o-proj partials
and the ReduceScatter also run in bf16).  Tricks:
  - K bias is dropped: softmax((q+bq)(k+bk)^T) == softmax((q+bq) k^T) since
    the (q+bq)*bk term is constant along the softmax (k) axis.  Q/V biases
    fold into the PSUM evacuations (per-partition scalar add / broadcast
    tile add); the o-proj bias folds into its evacuation or rank-1 PE
    matmuls for the Act-evacuated slices.
  - V is computed in natural [kpos, feature] layout directly (no transpose),
    with an appended ones column per head so PV accumulates the softmax
    denominators for free; the chunk-major V loop rides the incoming xv
    chunk DMAs.
  - Scores/exp/PV touch only the causally valid column range of each k tile;
    only the [128,128] triangular diagonal block needs a mask multiply (both
    heads of a pair share si-interleaved sc/pr tiles, so exp and the mask
    multiply cover two heads per instruction).
  - A PSUM bank may hold only ONE in-flight accumulation group (a group's
    first matmul clears has_written for the whole bank), so every psum tile
    here is bank-exclusive per group.
  - Just-in-time DMA: the shared transfer pool is FIFO, so weight/xv loads
    are gated behind 1-element marker copies with real RAW deps on the
    projection evacuations / late xk chunks; xq/xk chunks stream just in
    time.
  - Software pipeline: qbp0 scores pre-emitted to feed Act during the V
    pass; PVs lag scores per stage; o-proj half 0 interleaves into stage
    (1,0) so its ReduceScatter overlaps the rest of attention; half 1's
    first slices interleave with the final PVs via a split (per 512-column)
    softmax normalization.
"""

import numpy as np

B, S, D, H = 2, 2048, 1024, 16
DK = D // H  # 64
N_CORES = 8
FPC = 256  # features per core

_CACHE = {}


def _build_nc():
    import os as os_mod
    from contextlib import ExitStack

    import concourse.mybir as mybir
    import concourse.tile as tile
    from concourse import bacc

    F32 = mybir.dt.float32
    BF16 = mybir.dt.bfloat16
    Exp = mybir.ActivationFunctionType.Exp
    Copy = mybir.ActivationFunctionType.Copy
    Identity = mybir.ActivationFunctionType.Identity

    nc = bacc.Bacc("TRN2", target_bir_lowering=False, debug=False, num_devices=8)

    xq = nc.dram_tensor("xq", [D, S], BF16, kind="ExternalInput")
    xk = nc.dram_tensor("xk", [D, S], BF16, kind="ExternalInput")
    xv = nc.dram_tensor("xv", [D, S], BF16, kind="ExternalInput")
    wq = nc.dram_tensor("wq", [D, FPC], BF16, kind="ExternalInput")
    wk = nc.dram_tensor("wk", [D, FPC], BF16, kind="ExternalInput")
    wv = nc.dram_tensor("wv", [D, FPC], BF16, kind="ExternalInput")
    wo = nc.dram_tensor("wo", [FPC, D], BF16, kind="ExternalInput")
    bq = nc.dram_tensor("bq", [128, 2], F32, kind="ExternalInput")
    bv = nc.dram_tensor("bv", [128, FPC], BF16, kind="ExternalInput")
    bo4 = nc.dram_tensor("bo4", [128, D], BF16, kind="ExternalInput")
    mtri = nc.dram_tensor("mtri", [128, 2, 128], BF16, kind="ExternalInput")
    out = nc.dram_tensor("out", [512, D], BF16, kind="ExternalOutput")

    debug_taps = bool(os_mod.environ.get("BASS_DEBUG_TAPS"))
    if debug_taps:
        dbg_q = nc.dram_tensor("dbg_q", [128, 2 * S], BF16, kind="ExternalOutput")
        dbg_k = nc.dram_tensor("dbg_k", [128, 2 * S], BF16, kind="ExternalOutput")
        dbg_v = nc.dram_tensor("dbg_v", [128, 16 * 4 * 65], BF16, kind="ExternalOutput")
        dbg_c = nc.dram_tensor("dbg_c", [128, 2 * S], BF16, kind="ExternalOutput")

    with tile.TileContext(nc) as tc:
        with (
            tc.tile_pool(name="consts", bufs=1) as consts,
            tc.tile_pool(name="persist", bufs=1) as persist,
            tc.tile_pool(name="prs", bufs=21) as prs,
            tc.tile_pool(name="normp", bufs=3) as normp,
            tc.tile_pool(name="oout", bufs=4) as oout,
            tc.tile_pool(name="dram", bufs=1, space="DRAM") as dram,
        ):
            # ---- SBUF constants (wq/wk/xin are freed after phase 1a) ----
            ph1_stack = ExitStack()
            ph1 = ph1_stack.enter_context(tc.tile_pool(name="ph1", bufs=1))
            xin = ph1_stack.enter_context(tc.tile_pool(name="xin", bufs=3))
            wq_s = ph1.tile([128, 8, FPC], BF16, tag="wq")
            wk_s = ph1.tile([128, 8, FPC], BF16, tag="wk")
            wv_s = consts.tile([128, 8, FPC], BF16, tag="wv")
            wo_s = consts.tile([128, 2, D], BF16, tag="wo")
            bq_s = consts.tile([128, 2], F32, tag="bq")
            bv_s = consts.tile([128, FPC], BF16, tag="bv")
            bo4_s = consts.tile([128, D], BF16, tag="bo4")
            mask_s = consts.tile([128, 2, 128], BF16, tag="mask")
            ones_s = consts.tile([1, 512], BF16, tag="ones")

            # ---- persistent activations ----
            # feature f of the core maps to (pt = f // 128, row = f % 128);
            # local head h lives at [64*(h%2) : 64*(h%2)+64, h//2, :]
            qT_s = persist.tile([128, 2, S], BF16, tag="qT")
            kT_s = persist.tile([128, 2, S], BF16, tag="kT")
            v_s = persist.tile([128, 16, 4, 65], BF16, tag="v")
            ctx_s = persist.tile([128, 2, S], BF16, tag="ctx")

            nc.vector.memset(ones_s[:], 1.0)
            nc.vector.memset(v_s[:, :, :, 64:65], 1.0)

            # ---- constant DMAs on the gpsimd queue ----
            # only what phase 1a needs up front; the rest is marker-gated so
            # it doesn't preempt the just-in-time xq/xk chunk streams in the
            # shared DMA pool
            gq = nc.gpsimd
            gq.dma_start(wq_s[:, 0:1, :], wq[0:128, :].rearrange(
                "(kc p) f -> p kc f", p=128))
            gq.dma_start(wq_s[:, 1:8, :], wq[128:1024, :].rearrange(
                "(kc p) f -> p kc f", p=128))
            gq.dma_start(bq_s[:], bq.ap())
            gq.dma_start(wk_s[:], wk.ap().rearrange("(kc p) f -> p kc f", p=128))

            def issue_gated_dmas(entries):
                # markers are 1-element DVE copies READING qT_s/kT_s, so they
                # get a real RAW dependency on the projection evacuations —
                # the scheduler cannot hoist these DMAs ahead of the jit
                # xq/xk chunk streams
                for marker, full, src, dep_ap in entries:
                    nc.vector.tensor_copy(marker, dep_ap)
                    gq.dma_start(full, src)

            def issue_v_const_dmas(dep_ap):
                issue_gated_dmas([
                    (wv_s[0:1, 0:1, 0:1], wv_s[:],
                     wv.ap().rearrange("(kc p) f -> p kc f", p=128), dep_ap),
                    (bv_s[0:1, 0:1], bv_s[:], bv.ap(), dep_ap),
                    (mask_s[0:1, 0:1, 0:1], mask_s[:], mtri.ap(), dep_ap),
                ])

            def issue_o_const_dmas():
                issue_gated_dmas([
                    (wo_s[0:1, 0:1, 0:1], wo_s[:],
                     wo.ap().rearrange("(fc p) d -> p fc d", p=128),
                     kT_s[0:1, 1, 1536:1537]),
                    (bo4_s[0:1, 0:1], bo4_s[:], bo4.ap(),
                     kT_s[0:1, 1, 1536:1537]),
                ])

            # xq chunk 0 split in 4 pieces so the first matmuls start early
            xt0 = xin.tile([128, S], BF16, tag="x", name="xt0")
            for qb in range(4):
                nc.sync.dma_start(
                    xt0[:, 512 * qb : 512 * (qb + 1)],
                    xq[0:128, 512 * qb : 512 * (qb + 1)],
                )

            # xv is resident for the natural-layout V pass
            xvp_stack = ExitStack()
            xvp = xvp_stack.enter_context(tc.tile_pool(name="xvp", bufs=1))
            xv_all = xvp.tile([128, 8, S], BF16, tag="xva")

            def issue_xv_dmas(dep_ap):
                # real RAW dependency on a late xk chunk landing keeps these
                # behind the jit xk stream in the FIFO transfer pool
                for kc in range(8):
                    nc.vector.tensor_copy(xv_all[0:1, kc, 0:1], dep_ap)
                    gq.dma_start(
                        xv_all[:, kc, :], xv[128 * kc : 128 * (kc + 1), :]
                    )

            # ---- phase 1a: Q and K projections (transposed layout) ----
            st1 = ExitStack()
            psP = st1.enter_context(tc.tile_pool(name="psP", bufs=8, space="PSUM"))

            def proj_pass(x_dram, w_tile, b_tile, dst, first):
                ps = {}
                xts = {}
                for kc in range(8):
                    if first and kc == 0:
                        xt = xt0
                    else:
                        xt = xin.tile([128, S], BF16, tag="x")
                        nc.sync.dma_start(xt[:], x_dram[128 * kc : 128 * (kc + 1), :])
                    xts[kc] = xt
                    for pt in range(2):
                        for qb in range(4):
                            if kc == 0:
                                ps[(pt, qb)] = psP.tile(
                                    [128, 512], F32, tag="pp", name=f"pp{pt}{qb}"
                                )
                            nc.tensor.matmul(
                                ps[(pt, qb)][:],
                                w_tile[:, kc, 128 * pt : 128 * (pt + 1)],
                                xt[:, 512 * qb : 512 * (qb + 1)],
                                start=(kc == 0),
                                stop=(kc == 7),
                            )
                for pt in range(2):
                    for qb in range(4):
                        # bias (per-partition in this transposed layout) is
                        # folded into the evacuation, which is split across
                        # DVE and Act so the next pass's PSUM buffers free
                        # up twice as fast
                        dst_ap = dst[:, pt, 512 * qb : 512 * (qb + 1)]
                        if qb % 2:
                            if b_tile is not None:
                                nc.scalar.activation(
                                    dst_ap, ps[(pt, qb)][:], Identity,
                                    bias=b_tile[:, pt : pt + 1],
                                )
                            else:
                                nc.scalar.activation(dst_ap, ps[(pt, qb)][:], Copy)
                        else:
                            if b_tile is not None:
                                nc.vector.tensor_scalar_add(
                                    dst_ap, ps[(pt, qb)][:], b_tile[:, pt : pt + 1]
                                )
                            else:
                                nc.vector.tensor_copy(dst_ap, ps[(pt, qb)][:])
                return xts

            proj_pass(xq, wq_s, bq_s, qT_s, first=True)
            xk_ts = proj_pass(xk, wk_s, None, kT_s, first=False)
            issue_v_const_dmas(xk_ts[5][0:1, 0:1])
            issue_xv_dmas(xk_ts[7][0:1, 0:1])
            issue_o_const_dmas()
            st1.close()

            # ---- attention state/helpers ----
            attn_stack = ExitStack()
            psS = attn_stack.enter_context(
                tc.tile_pool(name="psS", bufs=2, space="PSUM")
            )
            pr_map = {}
            ctx_map = {}

            def segments(s0):
                if s0 < 512:
                    return [(s0, 512), (512, 1024)]
                return [(s0, 1024)]

            def emit_scores(qbp, hp, ki):
                # both si heads of the pair share si-interleaved sc/pr tiles
                # so exp and the mask multiply cover two heads per
                # instruction; one sc tile per 512-column segment keeps the
                # PSUM footprint at 2 banks so bufs=2 still pipelines
                s0 = max(0, 128 * ki - 1024 * qbp)
                pr = prs.tile([128, 2, 1024], BF16, tag="pr", name=f"pr{qbp}{hp}_{ki}")
                for a, b in segments(s0):
                    sc = psS.tile(
                        [128, 2, 512], F32, tag="sc", name=f"sc{qbp}{hp}_{ki}_{a}"
                    )
                    for si in range(2):
                        nc.tensor.matmul(
                            sc[:, si, 0 : b - a],
                            kT_s[64 * si : 64 * si + 64, hp,
                                 128 * ki : 128 * (ki + 1)],
                            qT_s[64 * si : 64 * si + 64, hp,
                                 1024 * qbp + a : 1024 * qbp + b],
                            start=True,
                            stop=True,
                        )
                    nc.scalar.activation(
                        out=pr[:, :, a:b], in_=sc[:, :, 0 : b - a],
                        func=Exp, scale=0.125,
                    )
                if 128 * ki >= 1024 * qbp:  # diagonal tile inside this window
                    nc.vector.tensor_mul(
                        pr[:, :, s0 : s0 + 128], pr[:, :, s0 : s0 + 128], mask_s[:]
                    )
                pr_map[(qbp, hp, ki)] = (pr, s0)

            def emit_pv(qbp, hp, ki, psA):
                nkt = 8 * (qbp + 1)
                pr, s0 = pr_map.pop((qbp, hp, ki))
                last_a = (512 + 1024 * qbp) // 128 - 1
                for si in range(2):
                    key = (qbp, hp, si)
                    if key not in ctx_map:
                        ctx_map[key] = psA.tile(
                            [65, 1024], F32, tag="ctx", name=f"ctx{qbp}{hp}{si}"
                        )
                    ctx = ctx_map[key]
                    for a, b in segments(s0):
                        last = last_a if b == 512 else nkt - 1
                        nc.tensor.matmul(
                            ctx[:, a:b],
                            v_s[:, ki, 2 * hp + si, :],
                            pr[:, si, a:b],
                            start=(ki == 0),
                            stop=(ki == last),
                            skip_group_check=True,
                        )

            def emit_norm(qbp, hp, cols=(0, 1024), release=True):
                a, b = cols
                w = b - a
                ctmp, rc, rbc = {}, {}, {}
                for si in range(2):
                    ctx = ctx_map[(qbp, hp, si)]
                    if release and b == 1024:
                        ctx_map.pop((qbp, hp, si))
                    ctmp[si] = normp.tile([65, 1024], BF16, tag="ctmp", name=f"ctmp{si}")
                    nc.vector.tensor_copy(ctmp[si][:, 0:w], ctx[:, a:b])
                for si in range(2):
                    rc[si] = normp.tile([1, 1024], BF16, tag="rc", name=f"rc{si}")
                    with nc.allow_low_precision("softmax denom recip in bf16"):
                        nc.vector.reciprocal(rc[si][:, 0:w], ctmp[si][64:65, 0:w])
                for si in range(2):
                    rbc[si] = normp.tile([64, 1024], BF16, tag="rbc", name=f"rbc{si}")
                    nc.gpsimd.partition_broadcast(rbc[si][:, 0:w], rc[si][:, 0:w])
                for si in range(2):
                    nc.vector.tensor_mul(
                        ctx_s[64 * si : 64 * si + 64, hp,
                              1024 * qbp + a : 1024 * qbp + b],
                        ctmp[si][0:64, 0:w],
                        rbc[si][:, 0:w],
                    )

            rs_in = [dram.tile([S // 2, D], BF16, name=f"rs_in{i}") for i in range(2)]
            rs_out = [dram.tile([256, D], BF16, name=f"rs_out{i}") for i in range(2)]

            def emit_oproj_sl(h, sl, pool, evac, po_shape):
                st = 8 * h + sl
                po_t = pool.tile(po_shape, F32, tag="sc", name=f"po{h}_{sl}")
                three_d = len(po_shape) == 3
                pe_bias = evac == "act"
                for nb in range(2):
                    po_nb = po_t[:, nb, :] if three_d else po_t[:, 512 * nb : 512 * (nb + 1)]
                    for fc in range(2):
                        nc.tensor.matmul(
                            po_nb,
                            ctx_s[:, fc, 128 * st : 128 * (st + 1)],
                            wo_s[:, fc, 512 * nb : 512 * (nb + 1)],
                            start=(fc == 0),
                            stop=(fc == 1 and not pe_bias),
                        )
                    if pe_bias:
                        nc.tensor.matmul(
                            po_nb,
                            ones_s[0:1, 0:128],
                            bo4_s[0:1, 512 * nb : 512 * (nb + 1)],
                            start=False,
                            stop=True,
                            skip_group_check=True,
                        )
                ot = oout.tile([128, 1024], BF16, tag="ot")
                po_v = po_t[:] if three_d else po_t[:].rearrange("p (n x) -> p n x", n=2)
                ot_v = ot[:].rearrange("p (n x) -> p n x", n=2)
                if evac == "act":
                    nc.scalar.activation(ot_v, po_v, Copy)
                else:
                    # fold the b_o/4 bias into the PSUM evacuation
                    nc.vector.tensor_add(
                        ot_v, po_v, bo4_s[:].rearrange("p (n x) -> p n x", n=2)
                    )
                nc.sync.dma_start(rs_in[h][128 * sl : 128 * (sl + 1), :], ot[:])

            def emit_rs(h):
                if not os_mod.environ.get("BASS_SIM_NO_RS"):
                    import concourse.mybir as mybir_mod

                    nc.gpsimd.collective_compute(
                        "ReduceScatter",
                        mybir_mod.AluOpType.add,
                        replica_groups=[[0, 1, 2, 3], [4, 5, 6, 7]],
                        ins=[rs_in[h].opt()],
                        outs=[rs_out[h].opt()],
                    )
                    nc.sync.dma_start(
                        out[256 * h : 256 * (h + 1), :], rs_out[h][:]
                    )
                else:
                    nc.sync.dma_start(
                        out[256 * h : 256 * (h + 1), :], rs_in[h][0:256, :]
                    )

            # ---- phase 1b: qbp0-hp0 scores (PE/Act filler while the xv ----
            # ---- chunks stream in) + chunk-major natural-layout V pass ----
            for ki in range(8):
                emit_scores(0, 0, ki)
            for ki in range(8):
                emit_scores(0, 1, ki)
            stV = ExitStack()
            psV = stV.enter_context(tc.tile_pool(name="psV", bufs=4, space="PSUM"))
            # 4 single-bank tiles per group (one st each — a PSUM bank may
            # only hold ONE accumulation group at a time); chunk-major order
            # lets group 0 ride the incoming xv chunk DMAs
            for g in range(4):
                pvt = {}
                for kc in range(8):
                    for j in range(4):
                        st = 4 * g + j
                        if kc == 0:
                            pvt[j] = psV.tile(
                                [128, 512], F32, tag="pv", name=f"pv{g}_{j}"
                            )
                        nc.tensor.matmul(
                            pvt[j][:, 0:256],
                            xv_all[:, kc, 128 * st : 128 * (st + 1)],
                            wv_s[:, kc, :],
                            start=(kc == 0),
                            stop=(kc == 7),
                        )
                for j in range(4):
                    st = 4 * g + j
                    nc.vector.tensor_add(
                        v_s[:, st, :, 0:64],
                        pvt[j][:, 0:256].rearrange("p (h x) -> p h x", h=4),
                        bv_s[:].rearrange("p (h x) -> p h x", h=4),
                    )
            stV.close()
            xvp_stack.close()
            ph1_stack.close()

            psA = attn_stack.enter_context(
                tc.tile_pool(name="psA", bufs=2, space="PSUM")
            )

            # ---- stage (qbp0, hp1): hp0 PVs (scores pre-emitted),      ----
            # ---- interleaved with stage(1,0) scores to keep Act fed    ----
            for ki in range(8):
                emit_pv(0, 0, ki, psA)
                emit_scores(1, 0, ki)
            emit_norm(0, 0)

            # ---- stage (qbp1, hp0): scores + qbp0-hp1 PVs, then own  ----
            # ---- PVs + o-proj half0 as PE filler after norm(0,1)     ----
            # (ctx pool has 2 buffer pairs: (1,0)'s PVs may only start
            #  after norm(0,1) releases qbp0-hp1's ctx tiles)
            for ki in range(16):
                if ki >= 8:
                    emit_scores(1, 0, ki)
                if ki < 8:
                    emit_pv(0, 1, ki, psA)
                if ki == 8:
                    emit_norm(0, 1)
                if ki >= 9:
                    emit_pv(1, 0, ki - 9, psA)
                if ki >= 8:
                    emit_oproj_sl(0, ki - 8, psS, "dve", [128, 2, 512])
            for k in range(7, 16):
                emit_pv(1, 0, k, psA)
            emit_norm(1, 0)
            emit_rs(0)

            # ---- stage (qbp1, hp1): scores + own PVs (3-ki lag); ctx ----
            # ---- cols [0:512) finish at ki=11, so their norm + the   ----
            # ---- first o-proj half1 slices interleave into the tail  ----
            for ki in range(16):
                emit_scores(1, 1, ki)
                if ki >= 3:
                    emit_pv(1, 1, ki - 3, psA)
                if ki == 14:
                    emit_norm(1, 1, cols=(0, 512), release=False)
                if ki >= 15:
                    emit_oproj_sl(1, ki - 15, psS, "dve", [128, 2, 512])
            emit_pv(1, 1, 13, psA)
            emit_oproj_sl(1, 1, psS, "act", [128, 2, 512])
            emit_pv(1, 1, 14, psA)
            emit_oproj_sl(1, 2, psS, "dve", [128, 2, 512])
            emit_pv(1, 1, 15, psA)
            emit_oproj_sl(1, 3, psS, "act", [128, 2, 512])
            emit_norm(1, 1, cols=(512, 1024))
            for sl in range(4, 8):
                emit_oproj_sl(1, sl, psS, "act" if sl % 2 else "dve", [128, 2, 512])
            attn_stack.close()

            if debug_taps:
                nc.sync.dma_start(dbg_q.ap(), qT_s[:].rearrange("p a b -> p (a b)"))
                nc.sync.dma_start(dbg_k.ap(), kT_s[:].rearrange("p a b -> p (a b)"))
                nc.sync.dma_start(dbg_v.ap(), v_s[:].rearrange("p a b c -> p (a b c)"))
                nc.sync.dma_start(dbg_c.ap(), ctx_s[:].rearrange("p a b -> p (a b)"))

            # ---- final ReduceScatter ----
            emit_rs(1)

    nc.compile()
    return nc


def _prep_inputs(query, key_, value, w_q, b_q, w_k, b_k, w_v, b_v, w_o, b_o):
    """Build the 8 per-core input maps (host-side sharding / re-layout)."""
    import ml_dtypes

    bf16 = ml_dtypes.bfloat16
    f32 = np.float32

    r = np.arange(128)[:, None, None]
    j = np.arange(128)[None, None, :]
    # allowed iff q >= k on the diagonal tile; doubled for the si-pair layout
    mtri = np.broadcast_to(j >= r, (128, 2, 128)).astype(bf16)

    wqT = np.ascontiguousarray(np.asarray(w_q, f32).T)  # [D_in, D_out]
    wkT = np.ascontiguousarray(np.asarray(w_k, f32).T)
    wvT = np.ascontiguousarray(np.asarray(w_v, f32).T)
    woT = np.ascontiguousarray(np.asarray(w_o, f32).T)

    xT = {}
    for g in range(B):
        xT[("q", g)] = np.ascontiguousarray(np.asarray(query[g], f32).T.astype(bf16))
        xT[("k", g)] = np.ascontiguousarray(np.asarray(key_[g], f32).T.astype(bf16))
        xT[("v", g)] = np.ascontiguousarray(np.asarray(value[g], f32).T.astype(bf16))

    bo4 = np.broadcast_to(
        (np.asarray(b_o, f32) / 4.0).reshape(1, D), (128, D)
    ).astype(bf16)

    in_maps = []
    for c in range(N_CORES):
        g, p = c // 4, c % 4
        fsel = slice(FPC * p, FPC * (p + 1))
        in_maps.append({
            "xq": xT[("q", g)],
            "xk": xT[("k", g)],
            "xv": xT[("v", g)],
            "wq": np.ascontiguousarray(wqT[:, fsel].astype(bf16)),
            "wk": np.ascontiguousarray(wkT[:, fsel].astype(bf16)),
            "wv": np.ascontiguousarray(wvT[:, fsel].astype(bf16)),
            "wo": np.ascontiguousarray(woT[fsel, :].astype(bf16)),
            "bq": np.ascontiguousarray(
                np.asarray(b_q, f32)[fsel].reshape(2, 128).T),
            "bv": np.ascontiguousarray(np.broadcast_to(
                np.asarray(b_v, f32)[fsel], (128, FPC)).astype(bf16)),
            "bo4": bo4,
            "mtri": mtri,
        })
    return in_maps


def run(inputs, trace=False):
    from concourse.bass_utils import run_bass_kernel_spmd

    if "nc" not in _CACHE:
        _CACHE["nc"] = _build_nc()
    nc = _CACHE["nc"]
    in_maps = _prep_inputs(
        inputs["query"], inputs["key_"], inputs["value"],
        inputs["w_q"], inputs["b_q"], inputs["w_k"], inputs["b_k"],
        inputs["w_v"], inputs["b_v"], inputs["w_o"], inputs["b_o"],
    )
    res = run_bass_kernel_spmd(
        nc, in_maps, core_ids=list(range(N_CORES)), trace=trace,
    )
    out = np.empty((B, S, D), np.float32)
    for c in range(N_CORES):
        g, p = c // 4, c % 4
        # RS half i scatters q rows [1024*i + 256*p, 1024*i + 256*(p+1))
        o = np.asarray(res.results[c]["out"]).astype(np.float32)
        out[g, 256 * p : 256 * (p + 1), :] = o[0:256]
        out[g, 1024 + 256 * p : 1024 + 256 * (p + 1), :] = o[256:512]
    return out, res


def kernel(**inputs):
    out, _ = run(inputs, trace=False)
    return out


# revision 83
# speedup vs baseline: 1.0192x; 1.0192x over previous
"""Causal multi-head attention on 8 Trainium2 NeuronCores.

Sharding: core c -> (batch g = c // 4, head-group p = c % 4, heads 4p..4p+3).
Each core projects Q/K/V for its batch with its 256 feature columns
(column-sharded w_q/w_k/w_v), runs causal attention for its 4 heads, computes
the partial output projection with its 256 rows of w_o, and a ReduceScatter
over each batch group sums the partials.

All matmul operands are bf16 (fp32 PSUM accumulation; the o-proj partials
and the ReduceScatter also run in bf16).  Tricks:
  - K bias is dropped: softmax((q+bq)(k+bk)^T) == softmax((q+bq) k^T) since
    the (q+bq)*bk term is constant along the softmax (k) axis.  Q/V biases
    fold into the PSUM evacuations (per-partition scalar add / broadcast
    tile add); the o-proj bias folds into its evacuation or rank-1 PE
    matmuls for the Act-evacuated slices.
  - V is computed in natural [kpos, feature] layout directly (no transpose),
    with an appended ones column per head so PV accumulates the softmax
    denominators for free; the chunk-major V loop rides the incoming xv
    chunk DMAs.
  - Scores/exp/PV touch only the causally valid column range of each k tile;
    only the [128,128] triangular diagonal block needs a mask multiply (both
    heads of a pair share si-interleaved sc/pr tiles, so exp and the mask
    multiply cover two heads per instruction).
  - A PSUM bank may hold only ONE in-flight accumulation group (a group's
    first matmul clears has_written for the whole bank), so every psum tile
    here is bank-exclusive per group.
  - Just-in-time DMA: the shared transfer pool is FIFO, so weight/xv loads
    are gated behind 1-element marker copies with real RAW deps on the
    projection evacuations / late xk chunks; xq/xk chunks stream just in
    time.
  - Software pipeline: qbp0 scores pre-emitted to feed Act during the V
    pass; PVs lag scores per stage; o-proj half 0 interleaves into stage
    (1,0) so its ReduceScatter overlaps the rest of attention; half 1's
    first slices interleave with the final PVs via a split (per 512-column)
    softmax normalization.
"""

import numpy as np

B, S, D, H = 2, 2048, 1024, 16
DK = D // H  # 64
N_CORES = 8
FPC = 256  # features per core

_CACHE = {}


def _build_nc():
    import os as os_mod
    from contextlib import ExitStack

    import concourse.mybir as mybir
    import concourse.tile as tile
    from concourse import bacc

    F32 = mybir.dt.float32
    BF16 = mybir.dt.bfloat16
    Exp = mybir.ActivationFunctionType.Exp
    Copy = mybir.ActivationFunctionType.Copy
    Identity = mybir.ActivationFunctionType.Identity

    nc = bacc.Bacc("TRN2", target_bir_lowering=False, debug=False, num_devices=8)

    xq = nc.dram_tensor("xq", [D, S], BF16, kind="ExternalInput")
    xk = nc.dram_tensor("xk", [D, S], BF16, kind="ExternalInput")
    xv = nc.dram_tensor("xv", [D, S], BF16, kind="ExternalInput")
    wq = nc.dram_tensor("wq", [D, FPC], BF16, kind="ExternalInput")
    wk = nc.dram_tensor("wk", [D, FPC], BF16, kind="ExternalInput")
    wv = nc.dram_tensor("wv", [D, FPC], BF16, kind="ExternalInput")
    wo = nc.dram_tensor("wo", [FPC, D], BF16, kind="ExternalInput")
    bq = nc.dram_tensor("bq", [128, 2], F32, kind="ExternalInput")
    bv = nc.dram_tensor("bv", [128, FPC], BF16, kind="ExternalInput")
    bo4 = nc.dram_tensor("bo4", [128, D], BF16, kind="ExternalInput")
    mtri = nc.dram_tensor("mtri", [128, 2, 128], BF16, kind="ExternalInput")
    out = nc.dram_tensor("out", [512, D], BF16, kind="ExternalOutput")

    debug_taps = bool(os_mod.environ.get("BASS_DEBUG_TAPS"))
    if debug_taps:
        dbg_q = nc.dram_tensor("dbg_q", [128, 2 * S], BF16, kind="ExternalOutput")
        dbg_k = nc.dram_tensor("dbg_k", [128, 2 * S], BF16, kind="ExternalOutput")
        dbg_v = nc.dram_tensor("dbg_v", [128, 16 * 4 * 65], BF16, kind="ExternalOutput")
        dbg_c = nc.dram_tensor("dbg_c", [128, 2 * S], BF16, kind="ExternalOutput")

    with tile.TileContext(nc) as tc:
        with (
            tc.tile_pool(name="consts", bufs=1) as consts,
            tc.tile_pool(name="persist", bufs=1) as persist,
            tc.tile_pool(name="prs", bufs=21) as prs,
            tc.tile_pool(name="normp", bufs=3) as normp,
            tc.tile_pool(name="oout", bufs=4) as oout,
            tc.tile_pool(name="dram", bufs=1, space="DRAM") as dram,
        ):
            # ---- SBUF constants (wq/wk/xin are freed after phase 1a) ----
            ph1_stack = ExitStack()
            ph1 = ph1_stack.enter_context(tc.tile_pool(name="ph1", bufs=1))
            xin = ph1_stack.enter_context(tc.tile_pool(name="xin", bufs=3))
            wq_s = ph1.tile([128, 8, FPC], BF16, tag="wq")
            wk_s = ph1.tile([128, 8, FPC], BF16, tag="wk")
            wv_s = consts.tile([128, 8, FPC], BF16, tag="wv")
            wo_s = consts.tile([128, 2, D], BF16, tag="wo")
            bq_s = consts.tile([128, 2], F32, tag="bq")
            bv_s = consts.tile([128, FPC], BF16, tag="bv")
            bo4_s = consts.tile([128, D], BF16, tag="bo4")
            mask_s = consts.tile([128, 2, 128], BF16, tag="mask")
            ones_s = consts.tile([1, 512], BF16, tag="ones")

            # ---- persistent activations ----
            # feature f of the core maps to (pt = f // 128, row = f % 128);
            # local head h lives at [64*(h%2) : 64*(h%2)+64, h//2, :]
            qT_s = persist.tile([128, 2, S], BF16, tag="qT")
            kT_s = persist.tile([128, 2, S], BF16, tag="kT")
            v_s = persist.tile([128, 16, 4, 65], BF16, tag="v")
            ctx_s = persist.tile([128, 2, S], BF16, tag="ctx")

            nc.vector.memset(ones_s[:], 1.0)
            nc.vector.memset(v_s[:, :, :, 64:65], 1.0)

            # ---- constant DMAs on the gpsimd queue ----
            # only what phase 1a needs up front; the rest is marker-gated so
            # it doesn't preempt the just-in-time xq/xk chunk streams in the
            # shared DMA pool
            gq = nc.gpsimd
            gq.dma_start(wq_s[:, 0:1, :], wq[0:128, :].rearrange(
                "(kc p) f -> p kc f", p=128))
            gq.dma_start(wq_s[:, 1:8, :], wq[128:1024, :].rearrange(
                "(kc p) f -> p kc f", p=128))
            gq.dma_start(bq_s[:], bq.ap())
            gq.dma_start(wk_s[:], wk.ap().rearrange("(kc p) f -> p kc f", p=128))

            def issue_gated_dmas(entries):
                # markers are 1-element DVE copies READING qT_s/kT_s, so they
                # get a real RAW dependency on the projection evacuations —
                # the scheduler cannot hoist these DMAs ahead of the jit
                # xq/xk chunk streams
                for marker, full, src, dep_ap in entries:
                    nc.vector.tensor_copy(marker, dep_ap)
                    gq.dma_start(full, src)

            def issue_v_const_dmas(dep_ap):
                issue_gated_dmas([
                    (wv_s[0:1, 0:1, 0:1], wv_s[:],
                     wv.ap().rearrange("(kc p) f -> p kc f", p=128), dep_ap),
                    (bv_s[0:1, 0:1], bv_s[:], bv.ap(), dep_ap),
                    (mask_s[0:1, 0:1, 0:1], mask_s[:], mtri.ap(), dep_ap),
                ])

            def issue_o_const_dmas():
                issue_gated_dmas([
                    (wo_s[0:1, 0:1, 0:1], wo_s[:],
                     wo.ap().rearrange("(fc p) d -> p fc d", p=128),
                     kT_s[0:1, 1, 1536:1537]),
                    (bo4_s[0:1, 0:1], bo4_s[:], bo4.ap(),
                     kT_s[0:1, 1, 1536:1537]),
                ])

            # xq chunk 0 split in 4 pieces so the first matmuls start early
            xt0 = xin.tile([128, S], BF16, tag="x", name="xt0")
            for qb in range(4):
                nc.sync.dma_start(
                    xt0[:, 512 * qb : 512 * (qb + 1)],
                    xq[0:128, 512 * qb : 512 * (qb + 1)],
                )

            # xv is resident for the natural-layout V pass
            xvp_stack = ExitStack()
            xvp = xvp_stack.enter_context(tc.tile_pool(name="xvp", bufs=1))
            xv_all = xvp.tile([128, 8, S], BF16, tag="xva")

            def issue_xv_dmas(dep_ap):
                # real RAW dependency on a late xk chunk landing keeps these
                # behind the jit xk stream in the FIFO transfer pool
                for kc in range(8):
                    nc.vector.tensor_copy(xv_all[0:1, kc, 0:1], dep_ap)
                    gq.dma_start(
                        xv_all[:, kc, :], xv[128 * kc : 128 * (kc + 1), :]
                    )

            # ---- phase 1a: Q and K projections (transposed layout) ----
            st1 = ExitStack()
            psP = st1.enter_context(tc.tile_pool(name="psP", bufs=8, space="PSUM"))

            def proj_pass(x_dram, w_tile, b_tile, dst, first):
                ps = {}
                xts = {}
                for kc in range(8):
                    if first and kc == 0:
                        xt = xt0
                    else:
                        xt = xin.tile([128, S], BF16, tag="x")
                        nc.sync.dma_start(xt[:], x_dram[128 * kc : 128 * (kc + 1), :])
                    xts[kc] = xt
                    for pt in range(2):
                        for qb in range(4):
                            if kc == 0:
                                ps[(pt, qb)] = psP.tile(
                                    [128, 512], F32, tag="pp", name=f"pp{pt}{qb}"
                                )
                            nc.tensor.matmul(
                                ps[(pt, qb)][:],
                                w_tile[:, kc, 128 * pt : 128 * (pt + 1)],
                                xt[:, 512 * qb : 512 * (qb + 1)],
                                start=(kc == 0),
                                stop=(kc == 7),
                            )
                for pt in range(2):
                    for qb in range(4):
                        # bias (per-partition in this transposed layout) is
                        # folded into the evacuation, which is split across
                        # DVE and Act so the next pass's PSUM buffers free
                        # up twice as fast
                        dst_ap = dst[:, pt, 512 * qb : 512 * (qb + 1)]
                        if qb % 2:
                            if b_tile is not None:
                                nc.scalar.activation(
                                    dst_ap, ps[(pt, qb)][:], Identity,
                                    bias=b_tile[:, pt : pt + 1],
                                )
                            else:
                                nc.scalar.activation(dst_ap, ps[(pt, qb)][:], Copy)
                        else:
                            if b_tile is not None:
                                nc.vector.tensor_scalar_add(
                                    dst_ap, ps[(pt, qb)][:], b_tile[:, pt : pt + 1]
                                )
                            else:
                                nc.vector.tensor_copy(dst_ap, ps[(pt, qb)][:])
                return xts

            proj_pass(xq, wq_s, bq_s, qT_s, first=True)
            xk_ts = proj_pass(xk, wk_s, None, kT_s, first=False)
            issue_v_const_dmas(xk_ts[5][0:1, 0:1])
            issue_xv_dmas(xk_ts[7][0:1, 0:1])
            issue_o_const_dmas()
            st1.close()

            # ---- attention state/helpers ----
            attn_stack = ExitStack()
            psS = attn_stack.enter_context(
                tc.tile_pool(name="psS", bufs=2, space="PSUM")
            )
            pr_map = {}
            ctx_map = {}

            def segments(s0):
                if s0 < 512:
                    return [(s0, 512), (512, 1024)]
                return [(s0, 1024)]

            def emit_scores(qbp, hp, ki):
                # both si heads of the pair share si-interleaved sc/pr tiles
                # so exp and the mask multiply cover two heads per
                # instruction; one sc tile per 512-column segment keeps the
                # PSUM footprint at 2 banks so bufs=2 still pipelines
                s0 = max(0, 128 * ki - 1024 * qbp)
                pr = prs.tile([128, 2, 1024], BF16, tag="pr", name=f"pr{qbp}{hp}_{ki}")
                for a, b in segments(s0):
                    sc = psS.tile(
                        [128, 2, 512], F32, tag="sc", name=f"sc{qbp}{hp}_{ki}_{a}"
                    )
                    for si in range(2):
                        nc.tensor.matmul(
                            sc[:, si, 0 : b - a],
                            kT_s[64 * si : 64 * si + 64, hp,
                                 128 * ki : 128 * (ki + 1)],
                            qT_s[64 * si : 64 * si + 64, hp,
                                 1024 * qbp + a : 1024 * qbp + b],
                            start=True,
                            stop=True,
                        )
                    nc.scalar.activation(
                        out=pr[:, :, a:b], in_=sc[:, :, 0 : b - a],
                        func=Exp, scale=0.125,
                    )
                if 128 * ki >= 1024 * qbp:  # diagonal tile inside this window
                    nc.vector.tensor_mul(
                        pr[:, :, s0 : s0 + 128], pr[:, :, s0 : s0 + 128], mask_s[:]
                    )
                pr_map[(qbp, hp, ki)] = (pr, s0)

            def emit_pv(qbp, hp, ki, psA):
                nkt = 8 * (qbp + 1)
                pr, s0 = pr_map.pop((qbp, hp, ki))
                last_a = (512 + 1024 * qbp) // 128 - 1
                for si in range(2):
                    key = (qbp, hp, si)
                    if key not in ctx_map:
                        ctx_map[key] = psA.tile(
                            [65, 1024], F32, tag="ctx", name=f"ctx{qbp}{hp}{si}"
                        )
                    ctx = ctx_map[key]
                    for a, b in segments(s0):
                        last = last_a if b == 512 else nkt - 1
                        nc.tensor.matmul(
                            ctx[:, a:b],
                            v_s[:, ki, 2 * hp + si, :],
                            pr[:, si, a:b],
                            start=(ki == 0),
                            stop=(ki == last),
                            skip_group_check=True,
                        )

            def emit_norm(qbp, hp, cols=(0, 1024), release=True):
                a, b = cols
                w = b - a
                ctmp, rc, rbc = {}, {}, {}
                for si in range(2):
                    ctx = ctx_map[(qbp, hp, si)]
                    if release and b == 1024:
                        ctx_map.pop((qbp, hp, si))
                    ctmp[si] = normp.tile([65, 1024], BF16, tag="ctmp", name=f"ctmp{si}")
                    nc.vector.tensor_copy(ctmp[si][:, 0:w], ctx[:, a:b])
                for si in range(2):
                    rc[si] = normp.tile([1, 1024], BF16, tag="rc", name=f"rc{si}")
                    with nc.allow_low_precision("softmax denom recip in bf16"):
                        nc.vector.reciprocal(rc[si][:, 0:w], ctmp[si][64:65, 0:w])
                for si in range(2):
                    rbc[si] = normp.tile([64, 1024], BF16, tag="rbc", name=f"rbc{si}")
                    nc.gpsimd.partition_broadcast(rbc[si][:, 0:w], rc[si][:, 0:w])
                for si in range(2):
                    nc.vector.tensor_mul(
                        ctx_s[64 * si : 64 * si + 64, hp,
                              1024 * qbp + a : 1024 * qbp + b],
                        ctmp[si][0:64, 0:w],
                        rbc[si][:, 0:w],
                    )

            rs_in = [dram.tile([S // 2, D], BF16, name=f"rs_in{i}") for i in range(2)]
            rs_out = [dram.tile([256, D], BF16, name=f"rs_out{i}") for i in range(2)]

            def emit_oproj_sl(h, sl, pool, evac, po_shape):
                st = 8 * h + sl
                po_t = pool.tile(po_shape, F32, tag="sc", name=f"po{h}_{sl}")
                three_d = len(po_shape) == 3
                pe_bias = evac == "act"
                for nb in range(2):
                    po_nb = po_t[:, nb, :] if three_d else po_t[:, 512 * nb : 512 * (nb + 1)]
                    for fc in range(2):
                        nc.tensor.matmul(
                            po_nb,
                            ctx_s[:, fc, 128 * st : 128 * (st + 1)],
                            wo_s[:, fc, 512 * nb : 512 * (nb + 1)],
                            start=(fc == 0),
                            stop=(fc == 1 and not pe_bias),
                        )
                    if pe_bias:
                        nc.tensor.matmul(
                            po_nb,
                            ones_s[0:1, 0:128],
                            bo4_s[0:1, 512 * nb : 512 * (nb + 1)],
                            start=False,
                            stop=True,
                            skip_group_check=True,
                        )
                ot = oout.tile([128, 1024], BF16, tag="ot")
                po_v = po_t[:] if three_d else po_t[:].rearrange("p (n x) -> p n x", n=2)
                ot_v = ot[:].rearrange("p (n x) -> p n x", n=2)
                if evac == "act":
                    nc.scalar.activation(ot_v, po_v, Copy)
                else:
                    # fold the b_o/4 bias into the PSUM evacuation
                    nc.vector.tensor_add(
                        ot_v, po_v, bo4_s[:].rearrange("p (n x) -> p n x", n=2)
                    )
                nc.sync.dma_start(rs_in[h][128 * sl : 128 * (sl + 1), :], ot[:])

            def emit_rs(h):
                if not os_mod.environ.get("BASS_SIM_NO_RS"):
                    import concourse.mybir as mybir_mod

                    nc.gpsimd.collective_compute(
                        "ReduceScatter",
                        mybir_mod.AluOpType.add,
                        replica_groups=[[0, 1, 2, 3], [4, 5, 6, 7]],
                        ins=[rs_in[h].opt()],
                        outs=[rs_out[h].opt()],
                    )
                    nc.sync.dma_start(
                        out[256 * h : 256 * (h + 1), :], rs_out[h][:]
                    )
                else:
                    nc.sync.dma_start(
                        out[256 * h : 256 * (h + 1), :], rs_in[h][0:256, :]
                    )

            # ---- phase 1b: qbp0-hp0 scores (PE/Act filler while the xv ----
            # ---- chunks stream in) + chunk-major natural-layout V pass ----
            for ki in range(8):
                emit_scores(0, 0, ki)
            for ki in range(8):
                emit_scores(0, 1, ki)
            stV = ExitStack()
            psV = stV.enter_context(tc.tile_pool(name="psV", bufs=4, space="PSUM"))
            # 4 single-bank tiles per group (one st each — a PSUM bank may
            # only hold ONE accumulation group at a time); chunk-major order
            # lets group 0 ride the incoming xv chunk DMAs
            for g in range(4):
                pvt = {}
                for kc in range(8):
                    for j in range(4):
                        st = 4 * g + j
                        if kc == 0:
                            pvt[j] = psV.tile(
                                [128, 512], F32, tag="pv", name=f"pv{g}_{j}"
                            )
                        nc.tensor.matmul(
                            pvt[j][:, 0:256],
                            xv_all[:, kc, 128 * st : 128 * (st + 1)],
                            wv_s[:, kc, :],
                            start=(kc == 0),
                            stop=(kc == 7),
                        )
                for j in range(4):
                    st = 4 * g + j
                    nc.vector.tensor_add(
                        v_s[:, st, :, 0:64],
                        pvt[j][:, 0:256].rearrange("p (h x) -> p h x", h=4),
                        bv_s[:].rearrange("p (h x) -> p h x", h=4),
                    )
            stV.close()
            xvp_stack.close()
            ph1_stack.close()

            psA = attn_stack.enter_context(
                tc.tile_pool(name="psA", bufs=2, space="PSUM")
            )

            # ---- stage (qbp0, hp1): hp0 PVs (scores pre-emitted),      ----
            # ---- interleaved with stage(1,0) scores to keep Act fed    ----
            for ki in range(8):
                emit_pv(0, 0, ki, psA)
                emit_scores(1, 0, ki)
            emit_norm(0, 0)

            # ---- stage (qbp1, hp0): scores + qbp0-hp1 PVs, then own  ----
            # ---- PVs + o-proj half0 as PE filler after norm(0,1)     ----
            # (ctx pool has 2 buffer pairs: (1,0)'s PVs may only start
            #  after norm(0,1) releases qbp0-hp1's ctx tiles)
            for ki in range(16):
                if ki >= 8:
                    emit_scores(1, 0, ki)
                if ki < 8:
                    emit_pv(0, 1, ki, psA)
                if ki == 8:
                    emit_norm(0, 1)
                if ki >= 9:
                    emit_pv(1, 0, ki - 9, psA)
            # hp0-PV catchup doubles as o-proj half0 + feeds Act with the
            # first stage(1,1) scores so the exp pipeline never drains
            for k in range(7, 16):
                emit_pv(1, 0, k, psA)
                if k <= 14:
                    emit_oproj_sl(0, k - 7, psS, "dve", [128, 2, 512])
                emit_scores(1, 1, k - 7)
            emit_norm(1, 0)
            emit_rs(0)

            # ---- stage (qbp1, hp1): remaining scores + own PVs; ctx  ----
            # ---- cols [0:512) finish at ki=11, so their norm + the   ----
            # ---- first o-proj half1 slices interleave into the tail  ----
            for j in range(6):
                emit_pv(1, 1, j, psA)
            for ki in range(9, 16):
                emit_scores(1, 1, ki)
                emit_pv(1, 1, ki - 3, psA)
                if ki == 14:
                    emit_norm(1, 1, cols=(0, 512), release=False)
                if ki >= 15:
                    emit_oproj_sl(1, ki - 15, psS, "dve", [128, 2, 512])
            emit_pv(1, 1, 13, psA)
            emit_oproj_sl(1, 1, psS, "act", [128, 2, 512])
            emit_pv(1, 1, 14, psA)
            emit_oproj_sl(1, 2, psS, "dve", [128, 2, 512])
            emit_pv(1, 1, 15, psA)
            emit_oproj_sl(1, 3, psS, "act", [128, 2, 512])
            emit_norm(1, 1, cols=(512, 1024))
            for sl in range(4, 8):
                emit_oproj_sl(1, sl, psS, "act" if sl % 2 else "dve", [128, 2, 512])
            attn_stack.close()

            if debug_taps:
                nc.sync.dma_start(dbg_q.ap(), qT_s[:].rearrange("p a b -> p (a b)"))
                nc.sync.dma_start(dbg_k.ap(), kT_s[:].rearrange("p a b -> p (a b)"))
                nc.sync.dma_start(dbg_v.ap(), v_s[:].rearrange("p a b c -> p (a b c)"))
                nc.sync.dma_start(dbg_c.ap(), ctx_s[:].rearrange("p a b -> p (a b)"))

            # ---- final ReduceScatter ----
            emit_rs(1)

    nc.compile()
    return nc


def _prep_inputs(query, key_, value, w_q, b_q, w_k, b_k, w_v, b_v, w_o, b_o):
    """Build the 8 per-core input maps (host-side sharding / re-layout)."""
    import ml_dtypes

    bf16 = ml_dtypes.bfloat16
    f32 = np.float32

    r = np.arange(128)[:, None, None]
    j = np.arange(128)[None, None, :]
    # allowed iff q >= k on the diagonal tile; doubled for the si-pair layout
    mtri = np.broadcast_to(j >= r, (128, 2, 128)).astype(bf16)

    wqT = np.ascontiguousarray(np.asarray(w_q, f32).T)  # [D_in, D_out]
    wkT = np.ascontiguousarray(np.asarray(w_k, f32).T)
    wvT = np.ascontiguousarray(np.asarray(w_v, f32).T)
    woT = np.ascontiguousarray(np.asarray(w_o, f32).T)

    xT = {}
    for g in range(B):
        xT[("q", g)] = np.ascontiguousarray(np.asarray(query[g], f32).T.astype(bf16))
        xT[("k", g)] = np.ascontiguousarray(np.asarray(key_[g], f32).T.astype(bf16))
        xT[("v", g)] = np.ascontiguousarray(np.asarray(value[g], f32).T.astype(bf16))

    bo4 = np.broadcast_to(
        (np.asarray(b_o, f32) / 4.0).reshape(1, D), (128, D)
    ).astype(bf16)

    in_maps = []
    for c in range(N_CORES):
        g, p = c // 4, c % 4
        fsel = slice(FPC * p, FPC * (p + 1))
        in_maps.append({
            "xq": xT[("q", g)],
            "xk": xT[("k", g)],
            "xv": xT[("v", g)],
            "wq": np.ascontiguousarray(wqT[:, fsel].astype(bf16)),
            "wk": np.ascontiguousarray(wkT[:, fsel].astype(bf16)),
            "wv": np.ascontiguousarray(wvT[:, fsel].astype(bf16)),
            "wo": np.ascontiguousarray(woT[fsel, :].astype(bf16)),
            "bq": np.ascontiguousarray(
                np.asarray(b_q, f32)[fsel].reshape(2, 128).T),
            "bv": np.ascontiguousarray(np.broadcast_to(
                np.asarray(b_v, f32)[fsel], (128, FPC)).astype(bf16)),
            "bo4": bo4,
            "mtri": mtri,
        })
    return in_maps


def run(inputs, trace=False):
    from concourse.bass_utils import run_bass_kernel_spmd

    if "nc" not in _CACHE:
        _CACHE["nc"] = _build_nc()
    nc = _CACHE["nc"]
    in_maps = _prep_inputs(
        inputs["query"], inputs["key_"], inputs["value"],
        inputs["w_q"], inputs["b_q"], inputs["w_k"], inputs["b_k"],
        inputs["w_v"], inputs["b_v"], inputs["w_o"], inputs["b_o"],
    )
    res = run_bass_kernel_spmd(
        nc, in_maps, core_ids=list(range(N_CORES)), trace=trace,
    )
    out = np.empty((B, S, D), np.float32)
    for c in range(N_CORES):
        g, p = c // 4, c % 4
        # RS half i scatters q rows [1024*i + 256*p, 1024*i + 256*(p+1))
        o = np.asarray(res.results[c]["out"]).astype(np.float32)
        out[g, 256 * p : 256 * (p + 1), :] = o[0:256]
        out[g, 1024 + 256 * p : 1024 + 256 * (p + 1), :] = o[256:512]
    return out, res


def kernel(**inputs):
    out, _ = run(inputs, trace=False)
    return out


# revision 85
# speedup vs baseline: 1.0218x; 1.0025x over previous
"""Causal multi-head attention on 8 Trainium2 NeuronCores.

Sharding: core c -> (batch g = c // 4, head-group p = c % 4, heads 4p..4p+3).
Each core projects Q/K/V for its batch with its 256 feature columns
(column-sharded w_q/w_k/w_v), runs causal attention for its 4 heads, computes
the partial output projection with its 256 rows of w_o, and a ReduceScatter
over each batch group sums the partials.

All matmul operands are bf16 (fp32 PSUM accumulation; the o-proj partials
and the ReduceScatter also run in bf16).  Tricks:
  - K bias is dropped: softmax((q+bq)(k+bk)^T) == softmax((q+bq) k^T) since
    the (q+bq)*bk term is constant along the softmax (k) axis.  Q/V biases
    fold into the PSUM evacuations (per-partition scalar add / broadcast
    tile add); the o-proj bias folds into its evacuation or rank-1 PE
    matmuls for the Act-evacuated slices.
  - V is computed in natural [kpos, feature] layout directly (no transpose),
    with an appended ones column per head so PV accumulates the softmax
    denominators for free; the chunk-major V loop rides the incoming xv
    chunk DMAs.
  - Scores/exp/PV touch only the causally valid column range of each k tile;
    only the [128,128] triangular diagonal block needs a mask multiply (both
    heads of a pair share si-interleaved sc/pr tiles, so exp and the mask
    multiply cover two heads per instruction).
  - A PSUM bank may hold only ONE in-flight accumulation group (a group's
    first matmul clears has_written for the whole bank), so every psum tile
    here is bank-exclusive per group.
  - Just-in-time DMA: the shared transfer pool is FIFO, so weight/xv loads
    are gated behind 1-element marker copies with real RAW deps on the
    projection evacuations / late xk chunks; xq/xk chunks stream just in
    time.
  - Software pipeline: qbp0 scores pre-emitted to feed Act during the V
    pass; PVs lag scores per stage; o-proj half 0 interleaves into stage
    (1,0) so its ReduceScatter overlaps the rest of attention; half 1's
    first slices interleave with the final PVs via a split (per 512-column)
    softmax normalization.
"""

import numpy as np

B, S, D, H = 2, 2048, 1024, 16
DK = D // H  # 64
N_CORES = 8
FPC = 256  # features per core

_CACHE = {}


def _build_nc():
    import os as os_mod
    from contextlib import ExitStack

    import concourse.mybir as mybir
    import concourse.tile as tile
    from concourse import bacc

    F32 = mybir.dt.float32
    BF16 = mybir.dt.bfloat16
    Exp = mybir.ActivationFunctionType.Exp
    Copy = mybir.ActivationFunctionType.Copy
    Identity = mybir.ActivationFunctionType.Identity

    nc = bacc.Bacc("TRN2", target_bir_lowering=False, debug=False, num_devices=8)

    xq = nc.dram_tensor("xq", [D, S], BF16, kind="ExternalInput")
    xk = nc.dram_tensor("xk", [D, S], BF16, kind="ExternalInput")
    xv = nc.dram_tensor("xv", [D, S], BF16, kind="ExternalInput")
    wq = nc.dram_tensor("wq", [D, FPC], BF16, kind="ExternalInput")
    wk = nc.dram_tensor("wk", [D, FPC], BF16, kind="ExternalInput")
    wv = nc.dram_tensor("wv", [D, FPC], BF16, kind="ExternalInput")
    wo = nc.dram_tensor("wo", [FPC, D], BF16, kind="ExternalInput")
    bq = nc.dram_tensor("bq", [128, 2], F32, kind="ExternalInput")
    bv = nc.dram_tensor("bv", [128, FPC], BF16, kind="ExternalInput")
    bo4 = nc.dram_tensor("bo4", [128, D], BF16, kind="ExternalInput")
    mtri = nc.dram_tensor("mtri", [128, 2, 128], BF16, kind="ExternalInput")
    out = nc.dram_tensor("out", [512, D], BF16, kind="ExternalOutput")

    debug_taps = bool(os_mod.environ.get("BASS_DEBUG_TAPS"))
    if debug_taps:
        dbg_q = nc.dram_tensor("dbg_q", [128, 2 * S], BF16, kind="ExternalOutput")
        dbg_k = nc.dram_tensor("dbg_k", [128, 2 * S], BF16, kind="ExternalOutput")
        dbg_v = nc.dram_tensor("dbg_v", [128, 16 * 4 * 65], BF16, kind="ExternalOutput")
        dbg_c = nc.dram_tensor("dbg_c", [128, 2 * S], BF16, kind="ExternalOutput")

    with tile.TileContext(nc) as tc:
        with (
            tc.tile_pool(name="consts", bufs=1) as consts,
            tc.tile_pool(name="persist", bufs=1) as persist,
            tc.tile_pool(name="prs", bufs=21) as prs,
            tc.tile_pool(name="normp", bufs=3) as normp,
            tc.tile_pool(name="oout", bufs=4) as oout,
            tc.tile_pool(name="dram", bufs=1, space="DRAM") as dram,
        ):
            # ---- SBUF constants (wq/wk/xin are freed after phase 1a) ----
            ph1_stack = ExitStack()
            ph1 = ph1_stack.enter_context(tc.tile_pool(name="ph1", bufs=1))
            xin = ph1_stack.enter_context(tc.tile_pool(name="xin", bufs=3))
            wq_s = ph1.tile([128, 8, FPC], BF16, tag="wq")
            wk_s = ph1.tile([128, 8, FPC], BF16, tag="wk")
            wv_s = consts.tile([128, 8, FPC], BF16, tag="wv")
            wo_s = consts.tile([128, 2, D], BF16, tag="wo")
            bq_s = consts.tile([128, 2], F32, tag="bq")
            bv_s = consts.tile([128, FPC], BF16, tag="bv")
            bo4_s = consts.tile([128, D], BF16, tag="bo4")
            mask_s = consts.tile([128, 2, 128], BF16, tag="mask")
            ones_s = consts.tile([1, 512], BF16, tag="ones")

            # ---- persistent activations ----
            # feature f of the core maps to (pt = f // 128, row = f % 128);
            # local head h lives at [64*(h%2) : 64*(h%2)+64, h//2, :]
            qT_s = persist.tile([128, 2, S], BF16, tag="qT")
            kT_s = persist.tile([128, 2, S], BF16, tag="kT")
            v_s = persist.tile([128, 16, 4, 65], BF16, tag="v")
            ctx_s = persist.tile([128, 2, S], BF16, tag="ctx")

            nc.vector.memset(ones_s[:], 1.0)
            nc.vector.memset(v_s[:, :, :, 64:65], 1.0)

            # ---- constant DMAs on the gpsimd queue ----
            # only what phase 1a needs up front; the rest is marker-gated so
            # it doesn't preempt the just-in-time xq/xk chunk streams in the
            # shared DMA pool
            gq = nc.gpsimd
            gq.dma_start(wq_s[:, 0:1, :], wq[0:128, :].rearrange(
                "(kc p) f -> p kc f", p=128))
            gq.dma_start(wq_s[:, 1:8, :], wq[128:1024, :].rearrange(
                "(kc p) f -> p kc f", p=128))
            gq.dma_start(bq_s[:], bq.ap())
            gq.dma_start(wk_s[:], wk.ap().rearrange("(kc p) f -> p kc f", p=128))

            def issue_gated_dmas(entries):
                # markers are 1-element DVE copies READING qT_s/kT_s, so they
                # get a real RAW dependency on the projection evacuations —
                # the scheduler cannot hoist these DMAs ahead of the jit
                # xq/xk chunk streams
                for marker, full, src, dep_ap in entries:
                    nc.vector.tensor_copy(marker, dep_ap)
                    gq.dma_start(full, src)

            def issue_v_const_dmas(dep_ap):
                issue_gated_dmas([
                    (wv_s[0:1, 0:1, 0:1], wv_s[:],
                     wv.ap().rearrange("(kc p) f -> p kc f", p=128), dep_ap),
                    (bv_s[0:1, 0:1], bv_s[:], bv.ap(), dep_ap),
                    (mask_s[0:1, 0:1, 0:1], mask_s[:], mtri.ap(), dep_ap),
                ])

            def issue_o_const_dmas():
                issue_gated_dmas([
                    (wo_s[0:1, 0:1, 0:1], wo_s[:],
                     wo.ap().rearrange("(fc p) d -> p fc d", p=128),
                     kT_s[0:1, 1, 1536:1537]),
                    (bo4_s[0:1, 0:1], bo4_s[:], bo4.ap(),
                     kT_s[0:1, 1, 1536:1537]),
                ])

            # xq chunk 0 split in 4 pieces so the first matmuls start early
            xt0 = xin.tile([128, S], BF16, tag="x", name="xt0")
            for qb in range(4):
                nc.sync.dma_start(
                    xt0[:, 512 * qb : 512 * (qb + 1)],
                    xq[0:128, 512 * qb : 512 * (qb + 1)],
                )

            # xv is resident for the natural-layout V pass
            xvp_stack = ExitStack()
            xvp = xvp_stack.enter_context(tc.tile_pool(name="xvp", bufs=1))
            xv_all = xvp.tile([128, 8, S], BF16, tag="xva")

            def issue_xv_dmas(dep_ap):
                # real RAW dependency on a late xk chunk landing keeps these
                # behind the jit xk stream in the FIFO transfer pool
                for kc in range(8):
                    nc.vector.tensor_copy(xv_all[0:1, kc, 0:1], dep_ap)
                    gq.dma_start(
                        xv_all[:, kc, :], xv[128 * kc : 128 * (kc + 1), :]
                    )

            # ---- phase 1a: Q and K projections (transposed layout) ----
            st1 = ExitStack()
            psP = st1.enter_context(tc.tile_pool(name="psP", bufs=8, space="PSUM"))

            def proj_pass(x_dram, w_tile, b_tile, dst, first):
                ps = {}
                xts = {}
                for kc in range(8):
                    if first and kc == 0:
                        xt = xt0
                    else:
                        xt = xin.tile([128, S], BF16, tag="x")
                        nc.sync.dma_start(xt[:], x_dram[128 * kc : 128 * (kc + 1), :])
                    xts[kc] = xt
                    for pt in range(2):
                        for qb in range(4):
                            if kc == 0:
                                ps[(pt, qb)] = psP.tile(
                                    [128, 512], F32, tag="pp", name=f"pp{pt}{qb}"
                                )
                            nc.tensor.matmul(
                                ps[(pt, qb)][:],
                                w_tile[:, kc, 128 * pt : 128 * (pt + 1)],
                                xt[:, 512 * qb : 512 * (qb + 1)],
                                start=(kc == 0),
                                stop=(kc == 7),
                            )
                for pt in range(2):
                    for qb in range(4):
                        # bias (per-partition in this transposed layout) is
                        # folded into the evacuation, which is split across
                        # DVE and Act so the next pass's PSUM buffers free
                        # up twice as fast
                        dst_ap = dst[:, pt, 512 * qb : 512 * (qb + 1)]
                        if qb % 2:
                            if b_tile is not None:
                                nc.scalar.activation(
                                    dst_ap, ps[(pt, qb)][:], Identity,
                                    bias=b_tile[:, pt : pt + 1],
                                )
                            else:
                                nc.scalar.activation(dst_ap, ps[(pt, qb)][:], Copy)
                        else:
                            if b_tile is not None:
                                nc.vector.tensor_scalar_add(
                                    dst_ap, ps[(pt, qb)][:], b_tile[:, pt : pt + 1]
                                )
                            else:
                                nc.vector.tensor_copy(dst_ap, ps[(pt, qb)][:])
                return xts

            proj_pass(xq, wq_s, bq_s, qT_s, first=True)
            xk_ts = proj_pass(xk, wk_s, None, kT_s, first=False)
            issue_v_const_dmas(xk_ts[5][0:1, 0:1])
            issue_xv_dmas(xk_ts[7][0:1, 0:1])
            issue_o_const_dmas()
            st1.close()

            # ---- attention state/helpers ----
            attn_stack = ExitStack()
            psS = attn_stack.enter_context(
                tc.tile_pool(name="psS", bufs=2, space="PSUM")
            )
            pr_map = {}
            ctx_map = {}

            def segments(s0):
                if s0 < 512:
                    return [(s0, 512), (512, 1024)]
                return [(s0, 1024)]

            def emit_scores(qbp, hp, ki):
                # both si heads of the pair share si-interleaved sc/pr tiles
                # so exp and the mask multiply cover two heads per
                # instruction; one sc tile per 512-column segment keeps the
                # PSUM footprint at 2 banks so bufs=2 still pipelines
                s0 = max(0, 128 * ki - 1024 * qbp)
                pr = prs.tile([128, 2, 1024], BF16, tag="pr", name=f"pr{qbp}{hp}_{ki}")
                for a, b in segments(s0):
                    sc = psS.tile(
                        [128, 2, 512], F32, tag="sc", name=f"sc{qbp}{hp}_{ki}_{a}"
                    )
                    for si in range(2):
                        nc.tensor.matmul(
                            sc[:, si, 0 : b - a],
                            kT_s[64 * si : 64 * si + 64, hp,
                                 128 * ki : 128 * (ki + 1)],
                            qT_s[64 * si : 64 * si + 64, hp,
                                 1024 * qbp + a : 1024 * qbp + b],
                            start=True,
                            stop=True,
                        )
                    nc.scalar.activation(
                        out=pr[:, :, a:b], in_=sc[:, :, 0 : b - a],
                        func=Exp, scale=0.125,
                    )
                if 128 * ki >= 1024 * qbp:  # diagonal tile inside this window
                    nc.vector.tensor_mul(
                        pr[:, :, s0 : s0 + 128], pr[:, :, s0 : s0 + 128], mask_s[:]
                    )
                pr_map[(qbp, hp, ki)] = (pr, s0)

            def emit_pv(qbp, hp, ki, psA):
                nkt = 8 * (qbp + 1)
                pr, s0 = pr_map.pop((qbp, hp, ki))
                last_a = (512 + 1024 * qbp) // 128 - 1
                for si in range(2):
                    key = (qbp, hp, si)
                    if key not in ctx_map:
                        ctx_map[key] = psA.tile(
                            [65, 1024], F32, tag="ctx", name=f"ctx{qbp}{hp}{si}"
                        )
                    ctx = ctx_map[key]
                    for a, b in segments(s0):
                        last = last_a if b == 512 else nkt - 1
                        nc.tensor.matmul(
                            ctx[:, a:b],
                            v_s[:, ki, 2 * hp + si, :],
                            pr[:, si, a:b],
                            start=(ki == 0),
                            stop=(ki == last),
                            skip_group_check=True,
                        )

            def emit_norm(qbp, hp, cols=(0, 1024), release=True):
                a, b = cols
                w = b - a
                ctmp, rc, rbc = {}, {}, {}
                for si in range(2):
                    ctx = ctx_map[(qbp, hp, si)]
                    if release and b == 1024:
                        ctx_map.pop((qbp, hp, si))
                    ctmp[si] = normp.tile([65, 1024], BF16, tag="ctmp", name=f"ctmp{si}")
                    nc.vector.tensor_copy(ctmp[si][:, 0:w], ctx[:, a:b])
                for si in range(2):
                    rc[si] = normp.tile([1, 1024], BF16, tag="rc", name=f"rc{si}")
                    with nc.allow_low_precision("softmax denom recip in bf16"):
                        nc.vector.reciprocal(rc[si][:, 0:w], ctmp[si][64:65, 0:w])
                for si in range(2):
                    rbc[si] = normp.tile([64, 1024], BF16, tag="rbc", name=f"rbc{si}")
                    nc.gpsimd.partition_broadcast(rbc[si][:, 0:w], rc[si][:, 0:w])
                for si in range(2):
                    nc.vector.tensor_mul(
                        ctx_s[64 * si : 64 * si + 64, hp,
                              1024 * qbp + a : 1024 * qbp + b],
                        ctmp[si][0:64, 0:w],
                        rbc[si][:, 0:w],
                    )

            rs_in = [dram.tile([S // 2, D], BF16, name=f"rs_in{i}") for i in range(2)]
            rs_out = [dram.tile([256, D], BF16, name=f"rs_out{i}") for i in range(2)]

            def emit_oproj_sl(h, sl, pool, evac, po_shape):
                st = 8 * h + sl
                po_t = pool.tile(po_shape, F32, tag="sc", name=f"po{h}_{sl}")
                three_d = len(po_shape) == 3
                pe_bias = evac == "act"
                for nb in range(2):
                    po_nb = po_t[:, nb, :] if three_d else po_t[:, 512 * nb : 512 * (nb + 1)]
                    for fc in range(2):
                        nc.tensor.matmul(
                            po_nb,
                            ctx_s[:, fc, 128 * st : 128 * (st + 1)],
                            wo_s[:, fc, 512 * nb : 512 * (nb + 1)],
                            start=(fc == 0),
                            stop=(fc == 1 and not pe_bias),
                        )
                    if pe_bias:
                        nc.tensor.matmul(
                            po_nb,
                            ones_s[0:1, 0:128],
                            bo4_s[0:1, 512 * nb : 512 * (nb + 1)],
                            start=False,
                            stop=True,
                            skip_group_check=True,
                        )
                ot = oout.tile([128, 1024], BF16, tag="ot")
                po_v = po_t[:] if three_d else po_t[:].rearrange("p (n x) -> p n x", n=2)
                ot_v = ot[:].rearrange("p (n x) -> p n x", n=2)
                if evac == "act":
                    nc.scalar.activation(ot_v, po_v, Copy)
                else:
                    # fold the b_o/4 bias into the PSUM evacuation
                    nc.vector.tensor_add(
                        ot_v, po_v, bo4_s[:].rearrange("p (n x) -> p n x", n=2)
                    )
                nc.sync.dma_start(rs_in[h][128 * sl : 128 * (sl + 1), :], ot[:])

            def emit_rs(h):
                if not os_mod.environ.get("BASS_SIM_NO_RS"):
                    import concourse.mybir as mybir_mod

                    nc.gpsimd.collective_compute(
                        "ReduceScatter",
                        mybir_mod.AluOpType.add,
                        replica_groups=[[0, 1, 2, 3], [4, 5, 6, 7]],
                        ins=[rs_in[h].opt()],
                        outs=[rs_out[h].opt()],
                    )
                    nc.sync.dma_start(
                        out[256 * h : 256 * (h + 1), :], rs_out[h][:]
                    )
                else:
                    nc.sync.dma_start(
                        out[256 * h : 256 * (h + 1), :], rs_in[h][0:256, :]
                    )

            # ---- phase 1b: qbp0-hp0 scores (PE/Act filler while the xv ----
            # ---- chunks stream in) + chunk-major natural-layout V pass ----
            for ki in range(8):
                emit_scores(0, 0, ki)
            for ki in range(8):
                emit_scores(0, 1, ki)
            stV = ExitStack()
            psV = stV.enter_context(tc.tile_pool(name="psV", bufs=4, space="PSUM"))
            # 4 single-bank tiles per group (one st each — a PSUM bank may
            # only hold ONE accumulation group at a time); chunk-major order
            # lets group 0 ride the incoming xv chunk DMAs
            for g in range(4):
                pvt = {}
                for kc in range(8):
                    for j in range(4):
                        st = 4 * g + j
                        if kc == 0:
                            pvt[j] = psV.tile(
                                [128, 512], F32, tag="pv", name=f"pv{g}_{j}"
                            )
                        nc.tensor.matmul(
                            pvt[j][:, 0:256],
                            xv_all[:, kc, 128 * st : 128 * (st + 1)],
                            wv_s[:, kc, :],
                            start=(kc == 0),
                            stop=(kc == 7),
                        )
                for j in range(4):
                    st = 4 * g + j
                    nc.vector.tensor_add(
                        v_s[:, st, :, 0:64],
                        pvt[j][:, 0:256].rearrange("p (h x) -> p h x", h=4),
                        bv_s[:].rearrange("p (h x) -> p h x", h=4),
                    )
            stV.close()
            xvp_stack.close()
            ph1_stack.close()

            psA = attn_stack.enter_context(
                tc.tile_pool(name="psA", bufs=2, space="PSUM")
            )

            # ---- stage (qbp0, hp1): hp0 PVs (scores pre-emitted),      ----
            # ---- interleaved with stage(1,0) scores to keep Act fed    ----
            for ki in range(8):
                emit_pv(0, 0, ki, psA)
                emit_scores(1, 0, ki)
            emit_norm(0, 0)

            # ---- stage (qbp1, hp0): scores + qbp0-hp1 PVs, then own  ----
            # ---- PVs + o-proj half0 as PE filler after norm(0,1)     ----
            # (ctx pool has 2 buffer pairs: (1,0)'s PVs may only start
            #  after norm(0,1) releases qbp0-hp1's ctx tiles)
            for ki in range(16):
                if ki >= 8:
                    emit_scores(1, 0, ki)
                if ki < 8:
                    emit_pv(0, 1, ki, psA)
                if ki == 8:
                    emit_norm(0, 1)
                if ki >= 9:
                    emit_pv(1, 0, ki - 9, psA)
            # hp0-PV catchup doubles as o-proj half0 + feeds Act with the
            # first stage(1,1) scores so the exp pipeline never drains
            for k in range(7, 16):
                emit_pv(1, 0, k, psA)
                emit_scores(1, 1, k - 7)
                if k <= 14:
                    emit_oproj_sl(0, k - 7, psS, "dve", [128, 2, 512])
            emit_norm(1, 0)
            emit_rs(0)

            # ---- stage (qbp1, hp1): remaining scores + own PVs; ctx  ----
            # ---- cols [0:512) finish at ki=11, so their norm + the   ----
            # ---- first o-proj half1 slices interleave into the tail  ----
            for j in range(6):
                emit_pv(1, 1, j, psA)
            for ki in range(9, 16):
                emit_scores(1, 1, ki)
                emit_pv(1, 1, ki - 3, psA)
                if ki == 14:
                    emit_norm(1, 1, cols=(0, 512), release=False)
                if ki >= 15:
                    emit_oproj_sl(1, ki - 15, psS, "dve", [128, 2, 512])
            emit_pv(1, 1, 13, psA)
            emit_oproj_sl(1, 1, psS, "act", [128, 2, 512])
            emit_pv(1, 1, 14, psA)
            emit_oproj_sl(1, 2, psS, "dve", [128, 2, 512])
            emit_pv(1, 1, 15, psA)
            emit_oproj_sl(1, 3, psS, "act", [128, 2, 512])
            emit_norm(1, 1, cols=(512, 1024))
            for sl in range(4, 8):
                emit_oproj_sl(1, sl, psS, "act" if sl % 2 else "dve", [128, 2, 512])
            attn_stack.close()

            if debug_taps:
                nc.sync.dma_start(dbg_q.ap(), qT_s[:].rearrange("p a b -> p (a b)"))
                nc.sync.dma_start(dbg_k.ap(), kT_s[:].rearrange("p a b -> p (a b)"))
                nc.sync.dma_start(dbg_v.ap(), v_s[:].rearrange("p a b c -> p (a b c)"))
                nc.sync.dma_start(dbg_c.ap(), ctx_s[:].rearrange("p a b -> p (a b)"))

            # ---- final ReduceScatter ----
            emit_rs(1)

    nc.compile()
    return nc


def _prep_inputs(query, key_, value, w_q, b_q, w_k, b_k, w_v, b_v, w_o, b_o):
    """Build the 8 per-core input maps (host-side sharding / re-layout)."""
    import ml_dtypes

    bf16 = ml_dtypes.bfloat16
    f32 = np.float32

    r = np.arange(128)[:, None, None]
    j = np.arange(128)[None, None, :]
    # allowed iff q >= k on the diagonal tile; doubled for the si-pair layout
    mtri = np.broadcast_to(j >= r, (128, 2, 128)).astype(bf16)

    wqT = np.ascontiguousarray(np.asarray(w_q, f32).T)  # [D_in, D_out]
    wkT = np.ascontiguousarray(np.asarray(w_k, f32).T)
    wvT = np.ascontiguousarray(np.asarray(w_v, f32).T)
    woT = np.ascontiguousarray(np.asarray(w_o, f32).T)

    xT = {}
    for g in range(B):
        xT[("q", g)] = np.ascontiguousarray(np.asarray(query[g], f32).T.astype(bf16))
        xT[("k", g)] = np.ascontiguousarray(np.asarray(key_[g], f32).T.astype(bf16))
        xT[("v", g)] = np.ascontiguousarray(np.asarray(value[g], f32).T.astype(bf16))

    bo4 = np.broadcast_to(
        (np.asarray(b_o, f32) / 4.0).reshape(1, D), (128, D)
    ).astype(bf16)

    in_maps = []
    for c in range(N_CORES):
        g, p = c // 4, c % 4
        fsel = slice(FPC * p, FPC * (p + 1))
        in_maps.append({
            "xq": xT[("q", g)],
            "xk": xT[("k", g)],
            "xv": xT[("v", g)],
            "wq": np.ascontiguousarray(wqT[:, fsel].astype(bf16)),
            "wk": np.ascontiguousarray(wkT[:, fsel].astype(bf16)),
            "wv": np.ascontiguousarray(wvT[:, fsel].astype(bf16)),
            "wo": np.ascontiguousarray(woT[fsel, :].astype(bf16)),
            "bq": np.ascontiguousarray(
                np.asarray(b_q, f32)[fsel].reshape(2, 128).T),
            "bv": np.ascontiguousarray(np.broadcast_to(
                np.asarray(b_v, f32)[fsel], (128, FPC)).astype(bf16)),
            "bo4": bo4,
            "mtri": mtri,
        })
    return in_maps


def run(inputs, trace=False):
    from concourse.bass_utils import run_bass_kernel_spmd

    if "nc" not in _CACHE:
        _CACHE["nc"] = _build_nc()
    nc = _CACHE["nc"]
    in_maps = _prep_inputs(
        inputs["query"], inputs["key_"], inputs["value"],
        inputs["w_q"], inputs["b_q"], inputs["w_k"], inputs["b_k"],
        inputs["w_v"], inputs["b_v"], inputs["w_o"], inputs["b_o"],
    )
    res = run_bass_kernel_spmd(
        nc, in_maps, core_ids=list(range(N_CORES)), trace=trace,
    )
    out = np.empty((B, S, D), np.float32)
    for c in range(N_CORES):
        g, p = c // 4, c % 4
        # RS half i scatters q rows [1024*i + 256*p, 1024*i + 256*(p+1))
        o = np.asarray(res.results[c]["out"]).astype(np.float32)
        out[g, 256 * p : 256 * (p + 1), :] = o[0:256]
        out[g, 1024 + 256 * p : 1024 + 256 * (p + 1), :] = o[256:512]
    return out, res


def kernel(**inputs):
    out, _ = run(inputs, trace=False)
    return out
